# revision 12
# baseline (speedup 1.0000x reference)
"""Single-head causal attention on 8 TRN2 NeuronCores.

Sharding: 2 cores per batch element (B=4). Within a batch pair, core
parity p owns the interleaved 128-row t-blocks {2j+p : j=0..15}.
Interleaving makes the causal block structure identical across cores
(one SPMD program, per-core data) and balances the flash-attention
work exactly.

Program uniformity trick: the host swaps adjacent 128-column blocks of
x^T for odd-parity cores, so the program always
  - reads its q columns from even block positions 2j, and
  - computes, for slot j, scores against buffer s-blocks 0..2j+1.
On even cores buffer block == global block: position 2j is the
diagonal (triangle mask), 2j+1 is fully above the diagonal (zero
mask). On odd cores position 2j holds global block 2j+1 (diagonal ->
triangle), 2j+1 holds global 2j (fully valid -> ones). The program
applies maskA to position E-2 and maskB to E-1; only the mask DATA
differs per core.

Per-core dataflow (no on-device transposes anywhere):
  qT[32,2048], kT[32,4096]: Wq/Wk chunk stationary, x^T moving
  v: x^T chunk stationary, Wv moving -> vones[128, sb, 33] tiles with
     a constant 1.0 column appended
  scores^T[s,t]: kT-block stationary, qT-slot moving -> PSUM f32
  exp on ScalarE with fused 1/sqrt(32) scale (scores are bounded ~1.5
     for this input scale, so no max-subtraction is needed; masked
     positions are multiplied to exact 0 after exp)
  attn^T[33,t] += [v|1] stationary, exp moving  (row 32 accumulates
     the softmax denominator for free)
  denom -> PE-transpose [1,128]->[128,1] -> reciprocal
  out[t,512] = attnT[0:32] stationary, Wp moving; normalized by a
     per-partition tensor_scalar multiply on the way out of PSUM.

Host folds bv@Wp + bp into one post-kernel broadcast add (softmax
weights sum to 1, so attn == attn_v + bv exactly).
"""

import math
import sys

for _p in ("/opt/trn_rl_repo", "/opt/trn_rl_repo/concourse"):
    if _p not in sys.path:
        sys.path.insert(0, _p)

import ml_dtypes
import numpy as np

BF16 = ml_dtypes.bfloat16

B, T, D, H = 4, 4096, 512, 32
NSLOT = 16          # 128-row t-blocks per core
NSB = T // 128      # 32 s-blocks
WAVE = 8            # s-blocks per PSUM wave (2 banks)
SCALE = 1.0 / math.sqrt(32.0)

_CACHE = {}


def build_nc():
    import concourse.mybir as mybir
    import concourse.tile as tile
    from concourse import bacc

    dt = mybir.dt
    nc = bacc.Bacc("TRN2", target_bir_lowering=False, debug=False)

    xT = nc.dram_tensor("xT", [4, 128, T], dt.bfloat16, kind="ExternalInput").ap()
    wq = nc.dram_tensor("wq", [4, 128, H], dt.bfloat16, kind="ExternalInput").ap()
    wk = nc.dram_tensor("wk", [4, 128, H], dt.bfloat16, kind="ExternalInput").ap()
    wv = nc.dram_tensor("wv", [4, 128, H], dt.bfloat16, kind="ExternalInput").ap()
    wp = nc.dram_tensor("wp", [H, D], dt.bfloat16, kind="ExternalInput").ap()
    bq = nc.dram_tensor("bq", [H, 1], dt.float32, kind="ExternalInput").ap()
    bk = nc.dram_tensor("bk", [H, 1], dt.float32, kind="ExternalInput").ap()
    masks = nc.dram_tensor(
        "masks", [2, 128, 128], dt.bfloat16, kind="ExternalInput"
    ).ap()
    out = nc.dram_tensor(
        "out", [NSLOT * 128, D], dt.float32, kind="ExternalOutput"
    ).ap()

    with tile.TileContext(nc) as tc, tc.tile_pool(
        name="singles", bufs=1
    ) as singles, tc.tile_pool(name="exp_pool", bufs=3) as exp_pool, tc.tile_pool(
        name="attnT_pool", bufs=2
    ) as attnT_pool, tc.tile_pool(
        name="recip_pool", bufs=2
    ) as recip_pool, tc.tile_pool(name="out_pool", bufs=2) as out_pool:
        # ---- resident SBUF tensors -----------------------------------
        xT_sb = singles.tile([128, 4, T], dt.bfloat16)
        wq_sb = singles.tile([128, 4, H], dt.bfloat16)
        wk_sb = singles.tile([128, 4, H], dt.bfloat16)
        wv_sb = singles.tile([128, 4, H], dt.bfloat16)
        wp_sb = singles.tile([H, D], dt.bfloat16)
        bq_sb = singles.tile([H, 1], dt.float32)
        bk_sb = singles.tile([H, 1], dt.float32)
        maskA = singles.tile([128, 128], dt.bfloat16)
        maskB = singles.tile([128, 128], dt.bfloat16)
        qT = singles.tile([H, NSLOT * 128], dt.bfloat16)
        kT = singles.tile([H, T], dt.bfloat16)
        vones = singles.tile([128, NSB, H + 1], dt.bfloat16)
        # all-ones [H+1, 1]; the [H:H+1] slice serves as the identity for
        # the denominator transpose (same base partition as attnT row H)
        identH = singles.tile([H + 1, 1], dt.bfloat16)

        nc.sync.dma_start(out=wq_sb, in_=wq.rearrange("c p h -> p c h"))
        nc.sync.dma_start(out=wk_sb, in_=wk.rearrange("c p h -> p c h"))
        nc.sync.dma_start(out=wv_sb, in_=wv.rearrange("c p h -> p c h"))
        nc.sync.dma_start(out=wp_sb, in_=wp)
        nc.sync.dma_start(out=bq_sb, in_=bq)
        nc.sync.dma_start(out=bk_sb, in_=bk)
        nc.sync.dma_start(out=maskA, in_=masks[0])
        nc.sync.dma_start(out=maskB, in_=masks[1])
        # x^T loads, grouped by t-quarter so compute can start early
        for tq in range(4):
            tsl = slice(tq * (T // 4), (tq + 1) * (T // 4))
            for e in range(4):
                nc.sync.dma_start(out=xT_sb[:, e, tsl], in_=xT[e, :, tsl])

        nc.vector.memset(vones, 1.0)
        nc.vector.memset(identH, 1.0)

        # ---- phase 1: projections ------------------------------------
        with tc.tile_pool(name="ps_proj", bufs=2, space="PSUM") as ps_proj:
            # kT[32, 4096]: Wk chunks stationary, x^T moving (N=512)
            for tb in range(T // 512):
                ksl = slice(tb * 512, (tb + 1) * 512)
                kps = ps_proj.tile([H, 512], dt.float32, tag="kq")
                for e in range(4):
                    nc.tensor.matmul(
                        kps,
                        wk_sb[:, e, :],
                        xT_sb[:, e, ksl],
                        start=(e == 0),
                        stop=(e == 3),
                    )
                nc.vector.tensor_scalar_add(kT[:, ksl], kps, bk_sb)

            # qT[32, 2048]: own q columns live at even buffer blocks 2j;
            # batch 4 slots per matmul with a strided moving AP
            xq = xT_sb.rearrange("p c (g two b) -> p c g two b", two=2, b=128)
            for g in range(4):
                qps = ps_proj.tile([H, 512], dt.float32, tag="kq")
                for e in range(4):
                    nc.tensor.matmul(
                        qps,
                        wq_sb[:, e, :],
                        xq[:, e, 4 * g : 4 * g + 4, 0, :],
                        start=(e == 0),
                        stop=(e == 3),
                    )
                nc.vector.tensor_scalar_add(
                    qT[:, g * 512 : (g + 1) * 512], qps, bq_sb
                )

            # v blocks: x^T chunk stationary, Wv moving (N=32); 4
            # s-blocks share one PSUM bank, copied out in one DVE op
            for vg in range(NSB // 4):
                vps = ps_proj.tile([128, 4, H], dt.float32, tag="v")
                for i in range(4):
                    sb = 4 * vg + i
                    for e in range(4):
                        nc.tensor.matmul(
                            vps[:, i, :],
                            xT_sb[:, e, sb * 128 : (sb + 1) * 128],
                            wv_sb[:, e, :],
                            start=(e == 0),
                            stop=(e == 3),
                        )
                nc.vector.tensor_copy(vones[:, 4 * vg : 4 * vg + 4, 0:H], vps)

        # ---- phase 2: flash attention, s-block-outer -----------------
        # Slots are packed 4-wide (one 512-col moving operand per
        # matmul). Two passes of slot-group pairs keep PSUM at 8 banks:
        # scores [128,2,512] x2bufs (4) + 2 attn accumulators (2) +
        # den/oproj (2). s-outer order loads each kT/vones stationary
        # once per pass (96 LDWEIGHTS total instead of 544).
        with tc.tile_pool(
            name="ps_scores", bufs=2, space="PSUM"
        ) as ps_scores, tc.tile_pool(
            name="ps_attn", bufs=1, space="PSUM"
        ) as ps_attn, tc.tile_pool(name="ps_misc", bufs=1, space="PSUM") as ps_misc:
            for g_lo, g_hi in ((0, 1), (2, 3)):
                e_lo, e_hi = 8 * g_lo + 8, 8 * g_hi + 8
                acc = {
                    g: ps_attn.tile(
                        [H + 1, 512],
                        dt.float32,
                        tag=f"acc{g % 2}",
                        name=f"acc{g}",
                    )
                    for g in (g_lo, g_hi)
                }
                for sb in range(e_hi):
                    both = sb < e_lo
                    active = (g_lo, g_hi) if both else (g_hi,)
                    # band: the 8 s-blocks straddling group gb's diagonal;
                    # within them only cols >= (r//2)*128 are causally live
                    gb, r = divmod(sb, 8)
                    band = gb in active
                    i0 = (r // 2) * 128 if band else 0

                    def csl(g):
                        return slice(i0 if g == gb else 0, 512)

                    scps = ps_scores.tile([128, 2, 512], dt.float32)
                    for g in active:
                        nc.tensor.matmul(
                            scps[:, g - g_lo, csl(g)],
                            kT[:, sb * 128 : (sb + 1) * 128],
                            qT[:, g * 512 + csl(g).start : (g + 1) * 512],
                            start=True,
                            stop=True,
                        )
                    # one exp over the active flat column range
                    off = (0 if both else 512) + (i0 if gb == active[0] else 0)
                    expt = exp_pool.tile([128, 2, 512], dt.bfloat16)
                    scf = scps.rearrange("p a b -> p (a b)")
                    exf = expt.rearrange("p a b -> p (a b)")
                    nc.scalar.activation(
                        exf[:, off:1024],
                        scf[:, off:1024],
                        mybir.ActivationFunctionType.Exp,
                        scale=SCALE,
                    )
                    if band:
                        # single 128-col mask at the diagonal block:
                        # r even -> triangle, r odd -> parity mask
                        msl = slice(i0, i0 + 128)
                        nc.vector.tensor_mul(
                            expt[:, gb - g_lo, msl],
                            expt[:, gb - g_lo, msl],
                            maskA if r % 2 == 0 else maskB,
                        )
                    for g in active:
                        nc.tensor.matmul(
                            acc[g][:, csl(g)],
                            vones[:, sb, :],
                            expt[:, g - g_lo, csl(g)],
                            start=(sb == 0),
                            stop=(sb == 8 * g + 7),
                            skip_group_check=True,
                        )
                # group epilogues: normalize + out-projection per slot
                for g in (g_lo, g_hi):
                    attnT = attnT_pool.tile([H + 1, 512], dt.bfloat16)
                    nc.vector.tensor_copy(attnT, acc[g])
                    for i in range(4):
                        j = 4 * g + i
                        tsl = slice(i * 128, (i + 1) * 128)
                        dps = ps_misc.tile([128, 1], dt.bfloat16, tag="den")
                        nc.tensor.transpose(
                            dps, attnT[H : H + 1, tsl], identH[H : H + 1, :]
                        )
                        recip = recip_pool.tile([128, 1], dt.float32)
                        nc.vector.reciprocal(recip, dps)
                        ops = ps_misc.tile([128, D], dt.float32, tag="oproj")
                        nc.tensor.matmul(
                            ops, attnT[0:H, tsl], wp_sb, start=True, stop=True
                        )
                        osb = out_pool.tile([128, D], dt.float32)
                        nc.vector.tensor_scalar_mul(osb, ops, recip)
                        nc.sync.dma_start(
                            out=out[j * 128 : (j + 1) * 128, :], in_=osb
                        )

    nc.compile()
    return nc


def _get_nc():
    if "nc" not in _CACHE:
        _CACHE["nc"] = build_nc()
    return _CACHE["nc"]


def make_in_maps(x, Wq, bq, Wk, bk, Wv, bv, Wp, bp):
    """Build the 8 per-core input maps (host-side sharding)."""
    x = np.asarray(x, dtype=np.float32)
    tri = np.tril(np.ones((128, 128), dtype=np.float32)).T  # [s,t]: 1 iff s<=t
    wq_s = np.ascontiguousarray(
        np.asarray(Wq, np.float32).reshape(4, 128, H)
    ).astype(BF16)
    wk_s = np.ascontiguousarray(
        np.asarray(Wk, np.float32).reshape(4, 128, H)
    ).astype(BF16)
    wv_s = np.ascontiguousarray(
        np.asarray(Wv, np.float32).reshape(4, 128, H)
    ).astype(BF16)
    wp_s = np.asarray(Wp, np.float32).astype(BF16)
    bq_s = np.ascontiguousarray(np.asarray(bq, np.float32).reshape(H, 1))
    bk_s = np.ascontiguousarray(np.asarray(bk, np.float32).reshape(H, 1))

    in_maps = []
    for c in range(8):
        b, p = divmod(c, 2)
        xb = x[b]  # [T, D]
        if p == 1:
            xb = xb.reshape(T // 256, 2, 128, D)[:, ::-1].reshape(T, D)
        xT_c = np.ascontiguousarray(xb.T).astype(BF16).reshape(4, 128, T)
        if p == 0:
            m = np.stack([tri, np.zeros((128, 128), np.float32)])
        else:
            m = np.stack([tri, np.ones((128, 128), np.float32)])
        in_maps.append(
            {
                "xT": xT_c,
                "wq": wq_s,
                "wk": wk_s,
                "wv": wv_s,
                "wp": wp_s,
                "bq": bq_s,
                "bk": bk_s,
                "masks": m.astype(BF16),
            }
        )
    return in_maps


def assemble_out(results, bv, Wp, bp):
    """Gather per-core [2048, 512] outputs into [B, T, D] and fold biases."""
    out = np.empty((B, T, D), dtype=np.float32)
    for c in range(8):
        b, p = divmod(c, 2)
        oc = np.asarray(results[c]["out"]).reshape(NSLOT, 128, D)
        for j in range(NSLOT):
            g = 2 * j + p
            out[b, g * 128 : (g + 1) * 128, :] = oc[j]
    out += (
        np.asarray(bv, np.float32) @ np.asarray(Wp, np.float32)
        + np.asarray(bp, np.float32)
    )[None, None, :]
    return out


def run_axon_percore(nc, in_maps, n_cores=8):
    """Run the same single-core NEFF on n_cores axon devices.

    bass2jax.run_bass_via_pjrt's multi-core branch uses shard_map over
    an 8-device mesh; under the axon loopback relay that execution
    never completes (the global-comm coordinated launch hangs). The
    kernel is pure data-parallel (no collectives), so n_cores
    independent per-device jit calls are semantically identical; jax's
    async dispatch lets them run concurrently. The NEFF is compiled
    once (neuron cache folds the identical bass_exec HLO).
    """
    import jax
    import concourse.mybir as mybir
    from concourse import bass2jax

    bass2jax.install_neuronx_cc_hook()

    partition_name = (
        nc.partition_id_tensor.name if nc.partition_id_tensor else None
    )
    in_names = []
    out_names = []
    out_avals = []
    zero_outs = []
    for alloc in nc.m.functions[0].allocations:
        if not isinstance(alloc, mybir.MemoryLocationSet):
            continue
        name = alloc.memorylocations[0].name
        if alloc.kind == "ExternalInput":
            if name != partition_name:
                in_names.append(name)
        elif alloc.kind == "ExternalOutput":
            out_names.append(name)
            shape = tuple(alloc.tensor_shape)
            dtype = mybir.dt.np(alloc.dtype)
            out_avals.append(jax.core.ShapedArray(shape, dtype))
            zero_outs.append(np.zeros(shape, dtype))
    n_params = len(in_names)
    all_names = in_names + out_names
    if partition_name is not None:
        all_names = all_names + [partition_name]

    def _body(*args):
        operands = list(args)
        if partition_name is not None:
            operands.append(bass2jax.partition_id_tensor())
        outs = bass2jax._bass_exec_p.bind(
            *operands,
            out_avals=tuple(out_avals),
            in_names=tuple(all_names),
            out_names=tuple(out_names),
            lowering_input_output_aliases=(),
            sim_require_finite=True,
            sim_require_nnan=True,
            nc=nc,
        )
        return tuple(outs)

    donate = tuple(range(n_params, n_params + len(out_names)))
    f = jax.jit(_body, donate_argnums=donate, keep_unused=True)
    devices = jax.devices()[:n_cores]
    pending = []
    for c in range(n_cores):
        args = [
            jax.device_put(np.asarray(in_maps[c][k]), devices[c])
            for k in in_names
        ] + [jax.device_put(z, devices[c]) for z in zero_outs]
        pending.append(f(*args))
    return [
        {name: np.asarray(outs[i]) for i, name in enumerate(out_names)}
        for outs in pending
    ]


def kernel(x, Wq, bq, Wk, bk, Wv, bv, Wp, bp):
    from concourse import bass_utils
    from concourse._compat import axon_active

    nc = _get_nc()
    in_maps = make_in_maps(x, Wq, bq, Wk, bk, Wv, bv, Wp, bp)
    if axon_active():
        results = run_axon_percore(nc, in_maps)
    else:
        res = bass_utils.run_bass_kernel_spmd(
            nc, in_maps, core_ids=list(range(8))
        )
        results = res.results
    return assemble_out(results, bv, Wp, bp)
